# revision 21
# baseline (speedup 1.0000x reference)
"""Additive attention (B=8, Q=K=1024, D=H=64) on 8 TRN2 NeuronCores.

Sparse + load-balanced rewrite.  valid_lens masks most of K (exp(-1e6)=0
exactly), so only ceil(vl/128) k-tiles per batch carry attention mass.  The
valid (batch, k-tile, q-half) "bricks" are distributed across all 8 cores
(pattern: each core gets three runs of 4/3/2 consecutive tiles, each run
within one (batch, q-half)); every core emits raw PV partial sums plus the
softmax denominator row, and the host sums partials across cores and divides.

Scores use the separable odd-harmonic sine expansion of tanh (see _fit):
S = FA^T @ FB with contraction 64*2*4 = 512 on the TensorEngine.  Masked
softmax runs without max-subtraction (|S| <= ~6; the -1e6 mask bias
underflows exp to exactly 0).
"""

import numpy as np
import ml_dtypes

B, Q, K = 8, 1024, 1024
D, H = 64, 64
NEG = -1000000.0
W0 = 0.46
MULTS = (1, 3, 5)
MF = len(MULTS)

TK = 128          # k-tile size
QB = 512          # q-block size
GROUP_SIZES = (4, 3, 2)   # tiles per A-group slot
NT = sum(GROUP_SIZES)     # 9 tile slots per core
NG = len(GROUP_SIZES)
GOFF = [0, 4, 7, 9]
NCST = 4 + MF + NT        # scales/biases + scale_B + per-tile mask cols

_CACHE = {}


def _fit_coeffs():
    x = np.linspace(0, 12.5, 4001)
    tg = np.tanh(x)
    wts = np.sqrt(np.exp(-x ** 2 / (2 * 2.03)) + 1e-4)
    Phi = np.sin(np.outer(x, W0 * np.array(MULTS)))
    c = np.linalg.lstsq(Phi * wts[:, None], tg * wts, rcond=None)[0]
    return c.astype(np.float64)


SIN_C = _fit_coeffs()


def _build():
    import concourse.bass as bass
    import concourse.bacc as bacc
    import concourse.mybir as mybir
    from concourse.tile import TileContext

    f32 = mybir.dt.float32
    bf16 = mybir.dt.bfloat16
    AFT = mybir.ActivationFunctionType

    nc = bacc.Bacc()

    # all bf16 inputs packed into one tensor to minimize per-DMA HWDGE setup:
    # cols 0:128 = [wq2 (parts 0:64); wk2 (parts 64:128)]
    # cols 128:128+1536 = [qTg (parts 0:64); kTs+pad (parts 64:128)]
    # cols 1664:2312 = vaug (full 128 partitions, NT x 72)
    PK = 128 + NG * QB + NT * 72
    pk_d = nc.declare_dram_parameter("pk", [128, PK], bf16, isOutput=False)
    cst_d = nc.declare_dram_parameter("cst", [128, NCST], f32, isOutput=False)
    po_d = nc.declare_dram_parameter("po", [72, NG * QB], bf16, isOutput=True)
    VOFF = 128 + NG * QB

    BL = NT * TK  # 1152 B columns
    # B psum chunks (PSUM bank holds 512 f32 cols)
    BCH = [(0, 512), (512, 1024), (1024, BL)]
    # B recurrence column ranges: group0's tiles first, rest after
    BR = [(0, 512), (512, BL)]

    with TileContext(nc) as tc:
        with (
            tc.tile_pool(name="inp", bufs=1) as inp,
            tc.tile_pool(name="feat", bufs=1) as feat,
            tc.tile_pool(name="work", bufs=3) as work,
            tc.tile_pool(name="ptab", bufs=3) as ptab,
            tc.tile_pool(name="ps_f", bufs=2, space="PSUM") as ps_f,
            tc.tile_pool(name="ps_sc", bufs=4, space="PSUM") as ps_sc,
            tc.tile_pool(name="ps_pv", bufs=2, space="PSUM") as ps_pv,
        ):
            pk1 = inp.tile([128, 640], bf16)
            pk2 = inp.tile([128, PK - 640], bf16)
            cst = inp.tile([128, NCST], f32)
            nc.sync.dma_start(out=pk1[:], in_=pk_d[:, 0:640])
            nc.sync.dma_start(out=cst[:], in_=cst_d[:])
            warm = inp.tile([1, 8], f32)
            nc.scalar.activation(warm[:], nc.const_aps.tensor(0.0, (1, 8)), AFT.Sin)
            nc.sync.dma_start(out=pk2[:], in_=pk_d[:, 640:PK])
            wq2 = pk1[0:64, 0:128]
            wk2 = pk1[64:128, 0:128]

            def qTg(g):  # packed cols 128 + g*QB, split at DMA boundary 640
                return (pk1[0:64, 128:640] if g == 0
                        else pk2[0:64, (g - 1) * QB:g * QB])

            def kch(c0, c1):  # kTs chunk by global k-col range
                if c1 <= 512:
                    return pk1[64:128, 128 + c0:128 + c1]
                return pk2[64:128, c0 - 512:c1 - 512]

            def vaug(t):
                o = VOFF - 640 + t * 72
                return pk2[:, o:o + 72]

            lo, hi = slice(0, 64), slice(64, 128)

            # ---- recurrence from X1=[sin;cos] (A) or [cos;sin] (B) ----
            # sq holds (s1^2 or sh^2) per half depending on layout; C2d both
            # halves from the s1^2 half; X3=(C2d+-1)X1; X_{m+2}=C2d*X_m-X_{m-2}
            def recur(tag, Xt, cs, sin_lo, ppneg_col, phase):
                ve = nc.vector
                X1 = Xt[1]
                w = cs[1] - cs[0]
                sl, co = (lo, hi) if sin_lo else (hi, lo)
                if phase == 0:
                    sq = work.tile([128, 640], bf16, tag=f"{tag}sq", name=f"{tag}sq")
                    Xt["sq"] = sq
                    ve.tensor_mul(sq[:, :w], X1[:, cs[0]:cs[1]],
                                  X1[:, cs[0]:cs[1]])
                    ve.tensor_scalar(X1[co, cs[0]:cs[1]], sq[co, :w], -2.0, 1.0,
                                     mybir.AluOpType.mult, mybir.AluOpType.add)
                    return
                C2d = Xt["C2d"]
                if phase == 1:
                    sq = Xt["sq"]
                    ve.tensor_scalar(C2d[co, cs[0]:cs[1]], sq[sl, :w], -4.0, 2.0,
                                     mybir.AluOpType.mult, mybir.AluOpType.add)
                    ve.tensor_scalar(C2d[sl, cs[0]:cs[1]], sq[sl, :w], -4.0, 2.0,
                                     mybir.AluOpType.mult, mybir.AluOpType.add)
                    pm = work.tile([128, 640], bf16, tag=f"{tag}pm", name=f"{tag}pm")
                    ve.tensor_scalar(pm[:, :w], C2d[:, cs[0]:cs[1]],
                                     cst[:, ppneg_col:ppneg_col + 1], None,
                                     mybir.AluOpType.add)
                    ve.tensor_mul(Xt[3][:, cs[0]:cs[1]], pm[:, :w], X1[:, cs[0]:cs[1]])
                    return
                m = MULTS[phase]
                eng = nc.gpsimd if (phase == 2 and tag == "BA0") else ve
                tmp = work.tile([128, 640], bf16, tag=f"{tag}tmp", name=f"{tag}tmp")
                eng.tensor_mul(tmp[:, :w], C2d[:, cs[0]:cs[1]], Xt[m - 2][:, cs[0]:cs[1]])
                eng.tensor_sub(Xt[m][:, cs[0]:cs[1]], tmp[:, :w], Xt[m - 4][:, cs[0]:cs[1]])

            def alloc_X(tag, n):
                Xt = {}
                for key in [1, "C2d"] + list(range(3, MULTS[-1] + 1, 2)):
                    Xt[key] = feat.tile([128, n], bf16, tag=f"{tag}{key}", name=f"x{tag}{key}")
                return Xt

            XA = [alloc_X(f"A{g}", QB) for g in range(NG)]
            XBr = alloc_X("B", BL)
            XB = {m: feat.tile([128, BL], bf16, tag=f"fb{m}", name=f"fb{m}")
                  for m in MULTS}

            # feature matmul / Sin emit helpers; interleaved with score
            # groups below so PE starts scores as early as possible while
            # keeping ALL Sins before the single Exp-set preload
            def fmmA(g):
                ps = ps_f.tile([128, 512], f32, tag="fp", name=f"psA{g}")
                nc.tensor.matmul(ps[:], wq2[:], qTg(g),
                                 start=True, stop=True)
                return ps

            def fmmB(i):
                c0, c1 = BCH[i]
                ps = ps_f.tile([128, 512], f32, tag="fp", name=f"psB{c0}")
                nc.tensor.matmul(ps[:, :c1 - c0], wk2[:], kch(c0, c1),
                                 start=True, stop=True)
                return ps

            def sinA(g, ps):
                nc.scalar.activation(XA[g][1][:], ps[:], AFT.Sin,
                                     scale=cst[:, 0:1])

            def sinB(i, ps):
                c0, c1 = BCH[i]
                nc.scalar.activation(XBr[1][:, c0:c1], ps[:, :c1 - c0], AFT.Sin,
                                     scale=cst[:, 1:2])

            # recurrences + B scaling, harmonic-major so the m=1 score
            # matmuls can start after only a few DVE ops
            def scale_m(i, m, br, eng=None):
                (eng or nc.vector).tensor_scalar_mul(XB[m][:, br[0]:br[1]],
                                                     XBr[m][:, br[0]:br[1]],
                                                     cst[:, 4 + i:5 + i])

            def recur_pair(tagA, XAg, Xb, br, also=None):
                # phase 0: cos halves + m1 scale; `also` interleaves a second
                # A-side chain (same phases) right behind this one on DVE
                recur(tagA, XAg, (0, QB), True, 2, phase=0)
                if br is not None:
                    recur("B" + tagA, Xb, br, False, 3, phase=0)
                    scale_m(0, 1, br)
                if also is not None:
                    recur("A2", also, (0, QB), True, 2, phase=0)
                # phases 1..: X3, X5(, X7) per side + scale
                for ph, (i, m) in zip(range(1, MF), list(enumerate(MULTS))[1:]):
                    recur(tagA, XAg, (0, QB), True, 2, phase=ph)
                    if br is not None:
                        recur("B" + tagA, Xb, br, False, 3, phase=ph)
                        scale_m(i, m, br,
                                eng=nc.gpsimd if tagA == "A1" else None)
                    if also is not None:
                        recur("A2", also, (0, QB), True, 2, phase=ph)



            # ---- bricks: scores (harmonic-major) -> exp -> PV per group ----
            outs = work.tile([72, NG * QB], bf16, tag="outs", name="outs")

            def score_group(g):
                n = GROUP_SIZES[g]
                sts = [ps_sc.tile([128, 512], f32, tag="st", name=f"st{g}_{j}")
                       for j in range(n)]
                for i, m in enumerate(MULTS):
                    for j in range(n):
                        t = GOFF[g] + j
                        nc.tensor.matmul(
                            sts[j][:],
                            XB[m][:, t * TK:(t + 1) * TK],
                            XA[g][m][:],
                            start=(i == 0), stop=(i == MF - 1),
                        )
                return sts

            def finish_group(g, sts):
                n = GROUP_SIZES[g]
                pv = ps_pv.tile([72, 512], f32, tag="pv", name=f"pv{g}")
                for j in range(n):
                    t = GOFF[g] + j
                    pt = ptab.tile([128, 512], bf16, tag="pt", name="pt")
                    nc.scalar.activation(pt[:], sts[j][:], AFT.Exp,
                                         bias=cst[:, 4 + MF + t:5 + MF + t])
                    nc.tensor.matmul(pv[:], vaug(t), pt[:],
                                     start=(j == 0), stop=(j == n - 1))
                gs = slice(g * QB, (g + 1) * QB)
                if g == 0:
                    nc.scalar.copy(outs[:, gs], pv[:])
                else:
                    nc.vector.tensor_copy(outs[:, gs], pv[:])
                nc.sync.dma_start(out=po_d[:, gs], in_=outs[:, gs])

            # ---- interleaved emission schedule ----
            ps0 = fmmA(0); psb0 = fmmB(0); psb1 = fmmB(1)
            sinA(0, ps0); sinB(0, psb0); sinB(1, psb1)
            recur_pair("A0", XA[0], XBr, BR[0])
            ps1 = fmmA(1); psb2 = fmmB(2)
            sinA(1, ps1); sinB(2, psb2)
            sts0 = score_group(0)
            recur_pair("A1", XA[1], XBr, BR[1])
            ps2 = fmmA(2)
            sinA(2, ps2)
            # preload Exp set; reading the LAST Sin's output makes this
            # data-depend on it so the scheduler cannot hoist it between the
            # Sins (which would thrash the ACT table sets)
            nc.scalar.activation(warm[:], XA[NG - 1][1][0:1, 0:8], AFT.Exp)
            sts1 = score_group(1)
            recur_pair("A2", XA[2], None, None)
            finish_group(0, sts0)
            sts2 = score_group(2)
            finish_group(1, sts1)
            finish_group(2, sts2)

    nc.finalize()
    return nc


_DECOMP = {8: (4, 4), 7: (4, 3), 6: (4, 2), 5: (3, 2), 4: (4,), 3: (3,),
           2: (2,), 1: (2,)}


def _plan(valid_lens):
    """Decompose valid (b, qb) tile runs into 8 cores x runs of GROUP_SIZES.

    Returns per-core list of groups: (b, qb, [kt list]) with dummy
    (-1, 0, [-1...]) groups and padded tiles marked kt=-1."""
    pieces = []  # (piece_size_slot, b, qb, [kts])
    for b in range(B):
        nt = int(np.ceil(valid_lens[b] / TK))
        for qb in range(2):
            kts = list(range(nt))
            rem = nt
            parts = []
            while rem > 8:
                parts.append(4)
                rem -= 4
            parts.extend(_DECOMP[rem] if rem else ())
            pos = 0
            for p in parts:
                take = kts[pos:pos + p]
                pos += len(take)
                pieces.append([p, b, qb, take])

    cores = [[] for _ in range(8)]
    for sz in GROUP_SIZES:
        avail = [p for p in pieces if p[0] == sz]
        # also allow smaller leftover pieces into larger slots if short
        extra = sorted((p for p in pieces if 0 < p[0] < sz), key=lambda p: -p[0])
        slots = []
        for c in range(8):
            if avail:
                p = avail.pop()
            elif extra:
                p = extra.pop(0)
            else:
                p = None
            slots.append(p)
        for c, p in enumerate(slots):
            if p is None:
                cores[c].append((-1, 0, [-1] * sz))
            else:
                assert len(p[3]) <= sz, f"piece too large for slot: {p} > {sz}"
                cores[c].append((p[1], p[2], p[3] + [-1] * (sz - len(p[3]))))
                p[0] = 0  # consumed
    unused = [p for p in pieces if p[0] > 0]
    assert not unused, f"unassigned pieces: {unused}"
    return cores


def _prep_in_maps(queries, keys, values, valid_lens, w_v, plan):
    qT = np.ascontiguousarray(queries.transpose(0, 2, 1)).astype(ml_dtypes.bfloat16)
    kT = np.ascontiguousarray(keys.transpose(0, 2, 1)).astype(ml_dtypes.bfloat16)
    vb = values.astype(ml_dtypes.bfloat16)

    base_cst = np.zeros((128, NCST), dtype=np.float32)
    base_cst[:64, 0] = W0; base_cst[64:, 0] = W0 / 2
    base_cst[:64, 1] = W0 / 2; base_cst[64:, 1] = W0
    base_cst[:64, 2] = 1.0; base_cst[64:, 2] = -1.0
    base_cst[:64, 3] = -1.0; base_cst[64:, 3] = 1.0
    for i in range(MF):
        sc = (w_v * SIN_C[i]).astype(np.float32)
        base_cst[:64, 4 + i] = sc
        base_cst[64:, 4 + i] = sc

    in_maps = []
    for c in range(8):
        groups = plan[c]
        qTg = np.zeros((D, NG * QB), dtype=ml_dtypes.bfloat16)
        kTs = np.zeros((D, NT * TK), dtype=ml_dtypes.bfloat16)
        vaug = np.zeros((128, NT * 72), dtype=ml_dtypes.bfloat16)
        cst = base_cst.copy()
        cst[:, 4 + MF:] = NEG  # default: padded tiles fully masked
        for g, (b, qb, kts) in enumerate(groups):
            if b < 0:
                continue
            qTg[:, g * QB:(g + 1) * QB] = qT[b][:, qb * QB:(qb + 1) * QB]
            vl = int(valid_lens[b])
            for j, kt in enumerate(kts):
                t = GOFF[g] + j
                if kt < 0:
                    continue
                ks = slice(kt * TK, (kt + 1) * TK)
                kTs[:, t * TK:(t + 1) * TK] = kT[b][:, ks]
                vaug[:, t * 72:t * 72 + 64] = vb[b][ks, :]
                vaug[:, t * 72 + 64] = 1.0
                cst[:, 4 + MF + t] = np.where(
                    np.arange(kt * TK, (kt + 1) * TK) < vl, 0.0, NEG
                ).astype(np.float32)
        PK = 128 + NG * QB + NT * 72
        pk = np.zeros((128, PK), dtype=ml_dtypes.bfloat16)
        pk[0:64, 0:128] = _prep_in_maps._wq2
        pk[64:128, 0:128] = _prep_in_maps._wk2
        pk[0:64, 128:128 + NG * QB] = qTg
        pk[64:128, 128:128 + NT * TK] = kTs
        pk[:, 128 + NG * QB:] = vaug
        in_maps.append({"pk": pk, "cst": cst})
    return in_maps


def kernel(queries, keys, values, valid_lens, W_q, W_k, w_v):
    from concourse.bass_utils import run_bass_kernel_spmd

    _prep_in_maps._wq2 = np.hstack([W_q, W_q]).astype(ml_dtypes.bfloat16)
    _prep_in_maps._wk2 = np.hstack([W_k, W_k]).astype(ml_dtypes.bfloat16)

    plan = _plan(np.asarray(valid_lens))

    if "nc" not in _CACHE:
        _CACHE["nc"] = _build()
    nc = _CACHE["nc"]

    in_maps = _prep_in_maps(queries, keys, values, np.asarray(valid_lens),
                            np.asarray(w_v, dtype=np.float32), plan)
    res = run_bass_kernel_spmd(nc, in_maps, core_ids=list(range(8)))

    num = np.zeros((B, 2, 64, QB), dtype=np.float64)
    den = np.zeros((B, 2, 1, QB), dtype=np.float64)
    for c in range(8):
        po = np.asarray(res.results[c]["po"], dtype=np.float64)  # [72, NG*QB]
        for g, (b, qb, kts) in enumerate(plan[c]):
            if b < 0:
                continue
            sl = po[:, g * QB:(g + 1) * QB]
            num[b, qb] += sl[0:64]
            den[b, qb] += sl[64:65]
    out = num / den  # [B, 2, 64, QB]
    out = out.transpose(0, 1, 3, 2).reshape(B, Q, 64)
    return out.astype(values.dtype)


# revision 22
# speedup vs baseline: 1.0369x; 1.0369x over previous
"""Additive attention (B=8, Q=K=1024, D=H=64) on 8 TRN2 NeuronCores.

Sparse + load-balanced rewrite.  valid_lens masks most of K (exp(-1e6)=0
exactly), so only ceil(vl/128) k-tiles per batch carry attention mass.  The
valid (batch, k-tile, q-half) "bricks" are distributed across all 8 cores
(pattern: each core gets three runs of 4/3/2 consecutive tiles, each run
within one (batch, q-half)); every core emits raw PV partial sums plus the
softmax denominator row, and the host sums partials across cores and divides.

Scores use the separable odd-harmonic sine expansion of tanh (see _fit):
S = FA^T @ FB with contraction 64*2*4 = 512 on the TensorEngine.  Masked
softmax runs without max-subtraction (|S| <= ~6; the -1e6 mask bias
underflows exp to exactly 0).
"""

import numpy as np
import ml_dtypes

B, Q, K = 8, 1024, 1024
D, H = 64, 64
NEG = -1000000.0
W0 = 0.46
MULTS = (1, 3, 5)
MF = len(MULTS)

TK = 128          # k-tile size
QB = 512          # q-block size
GROUP_SIZES = (4, 3, 2)   # tiles per A-group slot
NT = sum(GROUP_SIZES)     # 9 tile slots per core
NG = len(GROUP_SIZES)
GOFF = [0, 4, 7, 9]
NCST = 4 + MF + NT        # scales/biases + scale_B + per-tile mask cols

_CACHE = {}


def _fit_coeffs():
    x = np.linspace(0, 12.5, 4001)
    tg = np.tanh(x)
    wts = np.sqrt(np.exp(-x ** 2 / (2 * 2.03)) + 1e-4)
    Phi = np.sin(np.outer(x, W0 * np.array(MULTS)))
    c = np.linalg.lstsq(Phi * wts[:, None], tg * wts, rcond=None)[0]
    return c.astype(np.float64)


SIN_C = _fit_coeffs()


def _build():
    import concourse.bass as bass
    import concourse.bacc as bacc
    import concourse.mybir as mybir
    from concourse.tile import TileContext

    f32 = mybir.dt.float32
    bf16 = mybir.dt.bfloat16
    AFT = mybir.ActivationFunctionType

    nc = bacc.Bacc()

    # all bf16 inputs packed into one tensor to minimize per-DMA HWDGE setup:
    # cols 0:128 = [wq2 (parts 0:64); wk2 (parts 64:128)]
    # cols 128:128+1536 = [qTg (parts 0:64); kTs+pad (parts 64:128)]
    # cols 1664:2312 = vaug (full 128 partitions, NT x 72)
    PK = 128 + NG * QB + NT * 72
    pk_d = nc.declare_dram_parameter("pk", [128, PK], bf16, isOutput=False)
    cst_d = nc.declare_dram_parameter("cst", [128, NCST], f32, isOutput=False)
    po_d = nc.declare_dram_parameter("po", [72, NG * QB], bf16, isOutput=True)
    VOFF = 128 + NG * QB

    BL = NT * TK  # 1152 B columns
    # B psum chunks (PSUM bank holds 512 f32 cols)
    BCH = [(0, 512), (512, 1024), (1024, BL)]
    # B recurrence column ranges: group0's tiles first, rest after
    BR = [(0, 512), (512, BL)]

    with TileContext(nc) as tc:
        with (
            tc.tile_pool(name="inp", bufs=1) as inp,
            tc.tile_pool(name="feat", bufs=1) as feat,
            tc.tile_pool(name="work", bufs=3) as work,
            tc.tile_pool(name="ptab", bufs=3) as ptab,
            tc.tile_pool(name="ps_f", bufs=2, space="PSUM") as ps_f,
            tc.tile_pool(name="ps_sc", bufs=4, space="PSUM") as ps_sc,
            tc.tile_pool(name="ps_pv", bufs=2, space="PSUM") as ps_pv,
        ):
            pk1 = inp.tile([128, 640], bf16)
            pk2 = inp.tile([128, PK - 640], bf16)
            cst = inp.tile([128, NCST], f32)
            nc.sync.dma_start(out=pk1[:], in_=pk_d[:, 0:640])
            nc.sync.dma_start(out=cst[:], in_=cst_d[:])
            warm = inp.tile([1, 8], f32)
            nc.scalar.activation(warm[:], nc.const_aps.tensor(0.0, (1, 8)), AFT.Sin)
            nc.sync.dma_start(out=pk2[:], in_=pk_d[:, 640:PK])
            wq2 = pk1[0:64, 0:128]
            wk2 = pk1[64:128, 0:128]

            def qTg(g):  # packed cols 128 + g*QB, split at DMA boundary 640
                return (pk1[0:64, 128:640] if g == 0
                        else pk2[0:64, (g - 1) * QB:g * QB])

            def kch(c0, c1):  # kTs chunk by global k-col range
                if c1 <= 512:
                    return pk1[64:128, 128 + c0:128 + c1]
                return pk2[64:128, c0 - 512:c1 - 512]

            def vaug(t):
                o = VOFF - 640 + t * 72
                return pk2[:, o:o + 72]

            lo, hi = slice(0, 64), slice(64, 128)

            # ---- recurrence from X1=[sin;cos] (A) or [cos;sin] (B) ----
            # sq holds (s1^2 or sh^2) per half depending on layout; C2d both
            # halves from the s1^2 half; X3=(C2d+-1)X1; X_{m+2}=C2d*X_m-X_{m-2}
            def recur(tag, Xt, cs, sin_lo, ppneg_col, phase):
                ve = nc.vector
                X1 = Xt[1]
                w = cs[1] - cs[0]
                sl, co = (lo, hi) if sin_lo else (hi, lo)
                if phase == 0:
                    sq = work.tile([128, 640], bf16, tag=f"{tag}sq", name=f"{tag}sq")
                    Xt["sq"] = sq
                    ve.tensor_mul(sq[:, :w], X1[:, cs[0]:cs[1]],
                                  X1[:, cs[0]:cs[1]])
                    ve.tensor_scalar(X1[co, cs[0]:cs[1]], sq[co, :w], -2.0, 1.0,
                                     mybir.AluOpType.mult, mybir.AluOpType.add)
                    return
                C2d = Xt["C2d"]
                if phase == 1:
                    sq = Xt["sq"]
                    ve.tensor_scalar(C2d[co, cs[0]:cs[1]], sq[sl, :w], -4.0, 2.0,
                                     mybir.AluOpType.mult, mybir.AluOpType.add)
                    ve.tensor_scalar(C2d[sl, cs[0]:cs[1]], sq[sl, :w], -4.0, 2.0,
                                     mybir.AluOpType.mult, mybir.AluOpType.add)
                    pm = work.tile([128, 640], bf16, tag=f"{tag}pm", name=f"{tag}pm")
                    ve.tensor_scalar(pm[:, :w], C2d[:, cs[0]:cs[1]],
                                     cst[:, ppneg_col:ppneg_col + 1], None,
                                     mybir.AluOpType.add)
                    ve.tensor_mul(Xt[3][:, cs[0]:cs[1]], pm[:, :w], X1[:, cs[0]:cs[1]])
                    return
                m = MULTS[phase]
                eng = ve
                tmp = work.tile([128, 640], bf16, tag=f"{tag}tmp", name=f"{tag}tmp")
                eng.tensor_mul(tmp[:, :w], C2d[:, cs[0]:cs[1]], Xt[m - 2][:, cs[0]:cs[1]])
                eng.tensor_sub(Xt[m][:, cs[0]:cs[1]], tmp[:, :w], Xt[m - 4][:, cs[0]:cs[1]])

            def alloc_X(tag, n):
                Xt = {}
                for key in [1, "C2d"] + list(range(3, MULTS[-1] + 1, 2)):
                    Xt[key] = feat.tile([128, n], bf16, tag=f"{tag}{key}", name=f"x{tag}{key}")
                return Xt

            XA = [alloc_X(f"A{g}", QB) for g in range(NG)]
            XBr = alloc_X("B", BL)
            XB = {m: feat.tile([128, BL], bf16, tag=f"fb{m}", name=f"fb{m}")
                  for m in MULTS}

            # feature matmul / Sin emit helpers; interleaved with score
            # groups below so PE starts scores as early as possible while
            # keeping ALL Sins before the single Exp-set preload
            def fmmA(g):
                ps = ps_f.tile([128, 512], f32, tag="fp", name=f"psA{g}")
                nc.tensor.matmul(ps[:], wq2[:], qTg(g),
                                 start=True, stop=True)
                return ps

            def fmmB(i):
                c0, c1 = BCH[i]
                ps = ps_f.tile([128, 512], f32, tag="fp", name=f"psB{c0}")
                nc.tensor.matmul(ps[:, :c1 - c0], wk2[:], kch(c0, c1),
                                 start=True, stop=True)
                return ps

            def sinA(g, ps):
                nc.scalar.activation(XA[g][1][:], ps[:], AFT.Sin,
                                     scale=cst[:, 0:1])

            def sinB(i, ps):
                c0, c1 = BCH[i]
                nc.scalar.activation(XBr[1][:, c0:c1], ps[:, :c1 - c0], AFT.Sin,
                                     scale=cst[:, 1:2])

            # recurrences + B scaling, harmonic-major so the m=1 score
            # matmuls can start after only a few DVE ops
            def scale_m(i, m, br, eng=None):
                (eng or nc.vector).tensor_scalar_mul(XB[m][:, br[0]:br[1]],
                                                     XBr[m][:, br[0]:br[1]],
                                                     cst[:, 4 + i:5 + i])

            def recur_pair(tagA, XAg, Xb, br, also=None):
                # phase 0: cos halves + m1 scale; `also` interleaves a second
                # A-side chain (same phases) right behind this one on DVE
                recur(tagA, XAg, (0, QB), True, 2, phase=0)
                if br is not None:
                    recur("B" + tagA, Xb, br, False, 3, phase=0)
                    scale_m(0, 1, br)
                if also is not None:
                    recur("A2", also, (0, QB), True, 2, phase=0)
                # phases 1..: X3, X5(, X7) per side + scale
                for ph, (i, m) in zip(range(1, MF), list(enumerate(MULTS))[1:]):
                    recur(tagA, XAg, (0, QB), True, 2, phase=ph)
                    if br is not None:
                        recur("B" + tagA, Xb, br, False, 3, phase=ph)
                        scale_m(i, m, br)
                    if also is not None:
                        recur("A2", also, (0, QB), True, 2, phase=ph)



            # ---- bricks: scores (harmonic-major) -> exp -> PV per group ----
            outs = work.tile([72, NG * QB], bf16, tag="outs", name="outs")

            def score_group(g):
                n = GROUP_SIZES[g]
                sts = [ps_sc.tile([128, 512], f32, tag="st", name=f"st{g}_{j}")
                       for j in range(n)]
                for i, m in enumerate(MULTS):
                    for j in range(n):
                        t = GOFF[g] + j
                        nc.tensor.matmul(
                            sts[j][:],
                            XB[m][:, t * TK:(t + 1) * TK],
                            XA[g][m][:],
                            start=(i == 0), stop=(i == MF - 1),
                        )
                return sts

            def finish_group(g, sts):
                n = GROUP_SIZES[g]
                pv = ps_pv.tile([72, 512], f32, tag="pv", name=f"pv{g}")
                for j in range(n):
                    t = GOFF[g] + j
                    pt = ptab.tile([128, 512], bf16, tag="pt", name="pt")
                    nc.scalar.activation(pt[:], sts[j][:], AFT.Exp,
                                         bias=cst[:, 4 + MF + t:5 + MF + t])
                    nc.tensor.matmul(pv[:], vaug(t), pt[:],
                                     start=(j == 0), stop=(j == n - 1))
                gs = slice(g * QB, (g + 1) * QB)
                if g == 0:
                    nc.scalar.copy(outs[:, gs], pv[:])
                else:
                    nc.vector.tensor_copy(outs[:, gs], pv[:])
                nc.sync.dma_start(out=po_d[:, gs], in_=outs[:, gs])

            # ---- interleaved emission schedule ----
            ps0 = fmmA(0); psb0 = fmmB(0); psb1 = fmmB(1)
            sinA(0, ps0); sinB(0, psb0); sinB(1, psb1)
            recur_pair("A0", XA[0], XBr, BR[0])
            ps1 = fmmA(1); psb2 = fmmB(2)
            sinA(1, ps1); sinB(2, psb2)
            sts0 = score_group(0)
            recur_pair("A1", XA[1], XBr, BR[1])
            ps2 = fmmA(2)
            sinA(2, ps2)
            # preload Exp set; reading the LAST Sin's output makes this
            # data-depend on it so the scheduler cannot hoist it between the
            # Sins (which would thrash the ACT table sets)
            nc.scalar.activation(warm[:], XA[NG - 1][1][0:1, 0:8], AFT.Exp)
            sts1 = score_group(1)
            recur_pair("A2", XA[2], None, None)
            finish_group(0, sts0)
            sts2 = score_group(2)
            finish_group(1, sts1)
            finish_group(2, sts2)

    nc.finalize()
    return nc


_DECOMP = {8: (4, 4), 7: (4, 3), 6: (4, 2), 5: (3, 2), 4: (4,), 3: (3,),
           2: (2,), 1: (2,)}


def _plan(valid_lens):
    """Decompose valid (b, qb) tile runs into 8 cores x runs of GROUP_SIZES.

    Returns per-core list of groups: (b, qb, [kt list]) with dummy
    (-1, 0, [-1...]) groups and padded tiles marked kt=-1."""
    pieces = []  # (piece_size_slot, b, qb, [kts])
    for b in range(B):
        nt = int(np.ceil(valid_lens[b] / TK))
        for qb in range(2):
            kts = list(range(nt))
            rem = nt
            parts = []
            while rem > 8:
                parts.append(4)
                rem -= 4
            parts.extend(_DECOMP[rem] if rem else ())
            pos = 0
            for p in parts:
                take = kts[pos:pos + p]
                pos += len(take)
                pieces.append([p, b, qb, take])

    cores = [[] for _ in range(8)]
    for sz in GROUP_SIZES:
        avail = [p for p in pieces if p[0] == sz]
        # also allow smaller leftover pieces into larger slots if short
        extra = sorted((p for p in pieces if 0 < p[0] < sz), key=lambda p: -p[0])
        slots = []
        for c in range(8):
            if avail:
                p = avail.pop()
            elif extra:
                p = extra.pop(0)
            else:
                p = None
            slots.append(p)
        for c, p in enumerate(slots):
            if p is None:
                cores[c].append((-1, 0, [-1] * sz))
            else:
                assert len(p[3]) <= sz, f"piece too large for slot: {p} > {sz}"
                cores[c].append((p[1], p[2], p[3] + [-1] * (sz - len(p[3]))))
                p[0] = 0  # consumed
    unused = [p for p in pieces if p[0] > 0]
    assert not unused, f"unassigned pieces: {unused}"
    return cores


def _prep_in_maps(queries, keys, values, valid_lens, w_v, plan):
    qT = np.ascontiguousarray(queries.transpose(0, 2, 1)).astype(ml_dtypes.bfloat16)
    kT = np.ascontiguousarray(keys.transpose(0, 2, 1)).astype(ml_dtypes.bfloat16)
    vb = values.astype(ml_dtypes.bfloat16)

    base_cst = np.zeros((128, NCST), dtype=np.float32)
    base_cst[:64, 0] = W0; base_cst[64:, 0] = W0 / 2
    base_cst[:64, 1] = W0 / 2; base_cst[64:, 1] = W0
    base_cst[:64, 2] = 1.0; base_cst[64:, 2] = -1.0
    base_cst[:64, 3] = -1.0; base_cst[64:, 3] = 1.0
    for i in range(MF):
        sc = (w_v * SIN_C[i]).astype(np.float32)
        base_cst[:64, 4 + i] = sc
        base_cst[64:, 4 + i] = sc

    in_maps = []
    for c in range(8):
        groups = plan[c]
        qTg = np.zeros((D, NG * QB), dtype=ml_dtypes.bfloat16)
        kTs = np.zeros((D, NT * TK), dtype=ml_dtypes.bfloat16)
        vaug = np.zeros((128, NT * 72), dtype=ml_dtypes.bfloat16)
        cst = base_cst.copy()
        cst[:, 4 + MF:] = NEG  # default: padded tiles fully masked
        for g, (b, qb, kts) in enumerate(groups):
            if b < 0:
                continue
            qTg[:, g * QB:(g + 1) * QB] = qT[b][:, qb * QB:(qb + 1) * QB]
            vl = int(valid_lens[b])
            for j, kt in enumerate(kts):
                t = GOFF[g] + j
                if kt < 0:
                    continue
                ks = slice(kt * TK, (kt + 1) * TK)
                kTs[:, t * TK:(t + 1) * TK] = kT[b][:, ks]
                vaug[:, t * 72:t * 72 + 64] = vb[b][ks, :]
                vaug[:, t * 72 + 64] = 1.0
                cst[:, 4 + MF + t] = np.where(
                    np.arange(kt * TK, (kt + 1) * TK) < vl, 0.0, NEG
                ).astype(np.float32)
        PK = 128 + NG * QB + NT * 72
        pk = np.zeros((128, PK), dtype=ml_dtypes.bfloat16)
        pk[0:64, 0:128] = _prep_in_maps._wq2
        pk[64:128, 0:128] = _prep_in_maps._wk2
        pk[0:64, 128:128 + NG * QB] = qTg
        pk[64:128, 128:128 + NT * TK] = kTs
        pk[:, 128 + NG * QB:] = vaug
        in_maps.append({"pk": pk, "cst": cst})
    return in_maps


def kernel(queries, keys, values, valid_lens, W_q, W_k, w_v):
    from concourse.bass_utils import run_bass_kernel_spmd

    _prep_in_maps._wq2 = np.hstack([W_q, W_q]).astype(ml_dtypes.bfloat16)
    _prep_in_maps._wk2 = np.hstack([W_k, W_k]).astype(ml_dtypes.bfloat16)

    plan = _plan(np.asarray(valid_lens))

    if "nc" not in _CACHE:
        _CACHE["nc"] = _build()
    nc = _CACHE["nc"]

    in_maps = _prep_in_maps(queries, keys, values, np.asarray(valid_lens),
                            np.asarray(w_v, dtype=np.float32), plan)
    res = run_bass_kernel_spmd(nc, in_maps, core_ids=list(range(8)))

    num = np.zeros((B, 2, 64, QB), dtype=np.float64)
    den = np.zeros((B, 2, 1, QB), dtype=np.float64)
    for c in range(8):
        po = np.asarray(res.results[c]["po"], dtype=np.float64)  # [72, NG*QB]
        for g, (b, qb, kts) in enumerate(plan[c]):
            if b < 0:
                continue
            sl = po[:, g * QB:(g + 1) * QB]
            num[b, qb] += sl[0:64]
            den[b, qb] += sl[64:65]
    out = num / den  # [B, 2, 64, QB]
    out = out.transpose(0, 1, 3, 2).reshape(B, Q, 64)
    return out.astype(values.dtype)


# revision 23
# speedup vs baseline: 1.0387x; 1.0018x over previous
"""Additive attention (B=8, Q=K=1024, D=H=64) on 8 TRN2 NeuronCores.

Sparse + load-balanced rewrite.  valid_lens masks most of K (exp(-1e6)=0
exactly), so only ceil(vl/128) k-tiles per batch carry attention mass.  The
valid (batch, k-tile, q-half) "bricks" are distributed across all 8 cores
(pattern: each core gets three runs of 4/3/2 consecutive tiles, each run
within one (batch, q-half)); every core emits raw PV partial sums plus the
softmax denominator row, and the host sums partials across cores and divides.

Scores use the separable odd-harmonic sine expansion of tanh (see _fit):
S = FA^T @ FB with contraction 64*2*4 = 512 on the TensorEngine.  Masked
softmax runs without max-subtraction (|S| <= ~6; the -1e6 mask bias
underflows exp to exactly 0).
"""

import numpy as np
import ml_dtypes

B, Q, K = 8, 1024, 1024
D, H = 64, 64
NEG = -1000000.0
W0 = 0.46
MULTS = (1, 3, 5)
MF = len(MULTS)

TK = 128          # k-tile size
QB = 512          # q-block size
GROUP_SIZES = (4, 3, 2)   # tiles per A-group slot
NT = sum(GROUP_SIZES)     # 9 tile slots per core
NG = len(GROUP_SIZES)
GOFF = [0, 4, 7, 9]
NCST = 4 + MF + NT        # scales/biases + scale_B + per-tile mask cols

_CACHE = {}


def _fit_coeffs():
    x = np.linspace(0, 12.5, 4001)
    tg = np.tanh(x)
    wts = np.sqrt(np.exp(-x ** 2 / (2 * 2.03)) + 1e-4)
    Phi = np.sin(np.outer(x, W0 * np.array(MULTS)))
    c = np.linalg.lstsq(Phi * wts[:, None], tg * wts, rcond=None)[0]
    return c.astype(np.float64)


SIN_C = _fit_coeffs()


def _build():
    import concourse.bass as bass
    import concourse.bacc as bacc
    import concourse.mybir as mybir
    from concourse.tile import TileContext

    f32 = mybir.dt.float32
    bf16 = mybir.dt.bfloat16
    AFT = mybir.ActivationFunctionType

    nc = bacc.Bacc()

    # all bf16 inputs packed into one tensor to minimize per-DMA HWDGE setup:
    # cols 0:128 = [wq2 (parts 0:64); wk2 (parts 64:128)]
    # cols 128:128+1536 = [qTg (parts 0:64); kTs+pad (parts 64:128)]
    # cols 1664:2312 = vaug (full 128 partitions, NT x 72)
    PK = 128 + NG * QB + NT * 72
    pk_d = nc.declare_dram_parameter("pk", [128, PK], bf16, isOutput=False)
    cst_d = nc.declare_dram_parameter("cst", [128, NCST], f32, isOutput=False)
    po_d = nc.declare_dram_parameter("po", [72, NG * QB], bf16, isOutput=True)
    VOFF = 128 + NG * QB

    BL = NT * TK  # 1152 B columns
    # B psum chunks (PSUM bank holds 512 f32 cols)
    BCH = [(0, 512), (512, 1024), (1024, BL)]
    # B recurrence column ranges: group0's tiles first, rest after
    BR = [(0, 512), (512, BL)]

    with TileContext(nc) as tc:
        with (
            tc.tile_pool(name="inp", bufs=1) as inp,
            tc.tile_pool(name="feat", bufs=1) as feat,
            tc.tile_pool(name="work", bufs=3) as work,
            tc.tile_pool(name="ptab", bufs=3) as ptab,
            tc.tile_pool(name="ps_f", bufs=2, space="PSUM") as ps_f,
            tc.tile_pool(name="ps_sc", bufs=5, space="PSUM") as ps_sc,
            tc.tile_pool(name="ps_pv", bufs=1, space="PSUM") as ps_pv,
        ):
            pk1 = inp.tile([128, 640], bf16)
            pk2 = inp.tile([128, PK - 640], bf16)
            cst = inp.tile([128, NCST], f32)
            nc.sync.dma_start(out=pk1[:], in_=pk_d[:, 0:640])
            nc.sync.dma_start(out=cst[:], in_=cst_d[:])
            warm = inp.tile([1, 8], f32)
            nc.scalar.activation(warm[:], nc.const_aps.tensor(0.0, (1, 8)), AFT.Sin)
            nc.sync.dma_start(out=pk2[:], in_=pk_d[:, 640:PK])
            wq2 = pk1[0:64, 0:128]
            wk2 = pk1[64:128, 0:128]

            def qTg(g):  # packed cols 128 + g*QB, split at DMA boundary 640
                return (pk1[0:64, 128:640] if g == 0
                        else pk2[0:64, (g - 1) * QB:g * QB])

            def kch(c0, c1):  # kTs chunk by global k-col range
                if c1 <= 512:
                    return pk1[64:128, 128 + c0:128 + c1]
                return pk2[64:128, c0 - 512:c1 - 512]

            def vaug(t):
                o = VOFF - 640 + t * 72
                return pk2[:, o:o + 72]

            lo, hi = slice(0, 64), slice(64, 128)

            # ---- recurrence from X1=[sin;cos] (A) or [cos;sin] (B) ----
            # sq holds (s1^2 or sh^2) per half depending on layout; C2d both
            # halves from the s1^2 half; X3=(C2d+-1)X1; X_{m+2}=C2d*X_m-X_{m-2}
            def recur(tag, Xt, cs, sin_lo, ppneg_col, phase):
                ve = nc.vector
                X1 = Xt[1]
                w = cs[1] - cs[0]
                sl, co = (lo, hi) if sin_lo else (hi, lo)
                if phase == 0:
                    sq = work.tile([128, 640], bf16, tag=f"{tag}sq", name=f"{tag}sq")
                    Xt["sq"] = sq
                    ve.tensor_mul(sq[:, :w], X1[:, cs[0]:cs[1]],
                                  X1[:, cs[0]:cs[1]])
                    ve.tensor_scalar(X1[co, cs[0]:cs[1]], sq[co, :w], -2.0, 1.0,
                                     mybir.AluOpType.mult, mybir.AluOpType.add)
                    return
                C2d = Xt["C2d"]
                if phase == 1:
                    sq = Xt["sq"]
                    ve.tensor_scalar(C2d[co, cs[0]:cs[1]], sq[sl, :w], -4.0, 2.0,
                                     mybir.AluOpType.mult, mybir.AluOpType.add)
                    ve.tensor_scalar(C2d[sl, cs[0]:cs[1]], sq[sl, :w], -4.0, 2.0,
                                     mybir.AluOpType.mult, mybir.AluOpType.add)
                    pm = work.tile([128, 640], bf16, tag=f"{tag}pm", name=f"{tag}pm")
                    ve.tensor_scalar(pm[:, :w], C2d[:, cs[0]:cs[1]],
                                     cst[:, ppneg_col:ppneg_col + 1], None,
                                     mybir.AluOpType.add)
                    ve.tensor_mul(Xt[3][:, cs[0]:cs[1]], pm[:, :w], X1[:, cs[0]:cs[1]])
                    return
                m = MULTS[phase]
                eng = ve
                tmp = work.tile([128, 640], bf16, tag=f"{tag}tmp", name=f"{tag}tmp")
                eng.tensor_mul(tmp[:, :w], C2d[:, cs[0]:cs[1]], Xt[m - 2][:, cs[0]:cs[1]])
                eng.tensor_sub(Xt[m][:, cs[0]:cs[1]], tmp[:, :w], Xt[m - 4][:, cs[0]:cs[1]])

            def alloc_X(tag, n):
                Xt = {}
                for key in [1, "C2d"] + list(range(3, MULTS[-1] + 1, 2)):
                    Xt[key] = feat.tile([128, n], bf16, tag=f"{tag}{key}", name=f"x{tag}{key}")
                return Xt

            XA = [alloc_X(f"A{g}", QB) for g in range(NG)]
            XBr = alloc_X("B", BL)
            XB = {m: feat.tile([128, BL], bf16, tag=f"fb{m}", name=f"fb{m}")
                  for m in MULTS}

            # feature matmul / Sin emit helpers; interleaved with score
            # groups below so PE starts scores as early as possible while
            # keeping ALL Sins before the single Exp-set preload
            def fmmA(g):
                ps = ps_f.tile([128, 512], f32, tag="fp", name=f"psA{g}")
                nc.tensor.matmul(ps[:], wq2[:], qTg(g),
                                 start=True, stop=True)
                return ps

            def fmmB(i):
                c0, c1 = BCH[i]
                ps = ps_f.tile([128, 512], f32, tag="fp", name=f"psB{c0}")
                nc.tensor.matmul(ps[:, :c1 - c0], wk2[:], kch(c0, c1),
                                 start=True, stop=True)
                return ps

            def sinA(g, ps):
                nc.scalar.activation(XA[g][1][:], ps[:], AFT.Sin,
                                     scale=cst[:, 0:1])

            def sinB(i, ps):
                c0, c1 = BCH[i]
                nc.scalar.activation(XBr[1][:, c0:c1], ps[:, :c1 - c0], AFT.Sin,
                                     scale=cst[:, 1:2])

            # recurrences + B scaling, harmonic-major so the m=1 score
            # matmuls can start after only a few DVE ops
            def scale_m(i, m, br, eng=None):
                (eng or nc.vector).tensor_scalar_mul(XB[m][:, br[0]:br[1]],
                                                     XBr[m][:, br[0]:br[1]],
                                                     cst[:, 4 + i:5 + i])

            def recur_pair(tagA, XAg, Xb, br, also=None):
                # phase 0: cos halves + m1 scale; `also` interleaves a second
                # A-side chain (same phases) right behind this one on DVE
                recur(tagA, XAg, (0, QB), True, 2, phase=0)
                if br is not None:
                    recur("B" + tagA, Xb, br, False, 3, phase=0)
                    scale_m(0, 1, br)
                if also is not None:
                    recur("A2", also, (0, QB), True, 2, phase=0)
                # phases 1..: X3, X5(, X7) per side + scale
                for ph, (i, m) in zip(range(1, MF), list(enumerate(MULTS))[1:]):
                    recur(tagA, XAg, (0, QB), True, 2, phase=ph)
                    if br is not None:
                        recur("B" + tagA, Xb, br, False, 3, phase=ph)
                        scale_m(i, m, br)
                    if also is not None:
                        recur("A2", also, (0, QB), True, 2, phase=ph)



            # ---- bricks: scores (harmonic-major) -> exp -> PV per group ----
            outs = work.tile([72, NG * QB], bf16, tag="outs", name="outs")

            def score_group(g):
                n = GROUP_SIZES[g]
                sts = [ps_sc.tile([128, 512], f32, tag="st", name=f"st{g}_{j}")
                       for j in range(n)]
                for i, m in enumerate(MULTS):
                    for j in range(n):
                        t = GOFF[g] + j
                        nc.tensor.matmul(
                            sts[j][:],
                            XB[m][:, t * TK:(t + 1) * TK],
                            XA[g][m][:],
                            start=(i == 0), stop=(i == MF - 1),
                        )
                return sts

            def finish_group(g, sts):
                n = GROUP_SIZES[g]
                pv = ps_pv.tile([72, 512], f32, tag="pv", name=f"pv{g}")
                for j in range(n):
                    t = GOFF[g] + j
                    pt = ptab.tile([128, 512], bf16, tag="pt", name="pt")
                    nc.scalar.activation(pt[:], sts[j][:], AFT.Exp,
                                         bias=cst[:, 4 + MF + t:5 + MF + t])
                    nc.tensor.matmul(pv[:], vaug(t), pt[:],
                                     start=(j == 0), stop=(j == n - 1))
                gs = slice(g * QB, (g + 1) * QB)
                if g == 0:
                    nc.scalar.copy(outs[:, gs], pv[:])
                else:
                    nc.vector.tensor_copy(outs[:, gs], pv[:])
                nc.sync.dma_start(out=po_d[:, gs], in_=outs[:, gs])

            # ---- interleaved emission schedule ----
            ps0 = fmmA(0); psb0 = fmmB(0); psb1 = fmmB(1)
            sinA(0, ps0); sinB(0, psb0); sinB(1, psb1)
            recur_pair("A0", XA[0], XBr, BR[0])
            ps1 = fmmA(1); psb2 = fmmB(2)
            sinA(1, ps1); sinB(2, psb2)
            sts0 = score_group(0)
            recur_pair("A1", XA[1], XBr, BR[1])
            ps2 = fmmA(2)
            sinA(2, ps2)
            # preload Exp set; reading the LAST Sin's output makes this
            # data-depend on it so the scheduler cannot hoist it between the
            # Sins (which would thrash the ACT table sets)
            nc.scalar.activation(warm[:], XA[NG - 1][1][0:1, 0:8], AFT.Exp)
            sts1 = score_group(1)
            recur_pair("A2", XA[2], None, None)
            finish_group(0, sts0)
            sts2 = score_group(2)
            finish_group(1, sts1)
            finish_group(2, sts2)

    nc.finalize()
    return nc


_DECOMP = {8: (4, 4), 7: (4, 3), 6: (4, 2), 5: (3, 2), 4: (4,), 3: (3,),
           2: (2,), 1: (2,)}


def _plan(valid_lens):
    """Decompose valid (b, qb) tile runs into 8 cores x runs of GROUP_SIZES.

    Returns per-core list of groups: (b, qb, [kt list]) with dummy
    (-1, 0, [-1...]) groups and padded tiles marked kt=-1."""
    pieces = []  # (piece_size_slot, b, qb, [kts])
    for b in range(B):
        nt = int(np.ceil(valid_lens[b] / TK))
        for qb in range(2):
            kts = list(range(nt))
            rem = nt
            parts = []
            while rem > 8:
                parts.append(4)
                rem -= 4
            parts.extend(_DECOMP[rem] if rem else ())
            pos = 0
            for p in parts:
                take = kts[pos:pos + p]
                pos += len(take)
                pieces.append([p, b, qb, take])

    cores = [[] for _ in range(8)]
    for sz in GROUP_SIZES:
        avail = [p for p in pieces if p[0] == sz]
        # also allow smaller leftover pieces into larger slots if short
        extra = sorted((p for p in pieces if 0 < p[0] < sz), key=lambda p: -p[0])
        slots = []
        for c in range(8):
            if avail:
                p = avail.pop()
            elif extra:
                p = extra.pop(0)
            else:
                p = None
            slots.append(p)
        for c, p in enumerate(slots):
            if p is None:
                cores[c].append((-1, 0, [-1] * sz))
            else:
                assert len(p[3]) <= sz, f"piece too large for slot: {p} > {sz}"
                cores[c].append((p[1], p[2], p[3] + [-1] * (sz - len(p[3]))))
                p[0] = 0  # consumed
    unused = [p for p in pieces if p[0] > 0]
    assert not unused, f"unassigned pieces: {unused}"
    return cores


def _prep_in_maps(queries, keys, values, valid_lens, w_v, plan):
    qT = np.ascontiguousarray(queries.transpose(0, 2, 1)).astype(ml_dtypes.bfloat16)
    kT = np.ascontiguousarray(keys.transpose(0, 2, 1)).astype(ml_dtypes.bfloat16)
    vb = values.astype(ml_dtypes.bfloat16)

    base_cst = np.zeros((128, NCST), dtype=np.float32)
    base_cst[:64, 0] = W0; base_cst[64:, 0] = W0 / 2
    base_cst[:64, 1] = W0 / 2; base_cst[64:, 1] = W0
    base_cst[:64, 2] = 1.0; base_cst[64:, 2] = -1.0
    base_cst[:64, 3] = -1.0; base_cst[64:, 3] = 1.0
    for i in range(MF):
        sc = (w_v * SIN_C[i]).astype(np.float32)
        base_cst[:64, 4 + i] = sc
        base_cst[64:, 4 + i] = sc

    in_maps = []
    for c in range(8):
        groups = plan[c]
        qTg = np.zeros((D, NG * QB), dtype=ml_dtypes.bfloat16)
        kTs = np.zeros((D, NT * TK), dtype=ml_dtypes.bfloat16)
        vaug = np.zeros((128, NT * 72), dtype=ml_dtypes.bfloat16)
        cst = base_cst.copy()
        cst[:, 4 + MF:] = NEG  # default: padded tiles fully masked
        for g, (b, qb, kts) in enumerate(groups):
            if b < 0:
                continue
            qTg[:, g * QB:(g + 1) * QB] = qT[b][:, qb * QB:(qb + 1) * QB]
            vl = int(valid_lens[b])
            for j, kt in enumerate(kts):
                t = GOFF[g] + j
                if kt < 0:
                    continue
                ks = slice(kt * TK, (kt + 1) * TK)
                kTs[:, t * TK:(t + 1) * TK] = kT[b][:, ks]
                vaug[:, t * 72:t * 72 + 64] = vb[b][ks, :]
                vaug[:, t * 72 + 64] = 1.0
                cst[:, 4 + MF + t] = np.where(
                    np.arange(kt * TK, (kt + 1) * TK) < vl, 0.0, NEG
                ).astype(np.float32)
        PK = 128 + NG * QB + NT * 72
        pk = np.zeros((128, PK), dtype=ml_dtypes.bfloat16)
        pk[0:64, 0:128] = _prep_in_maps._wq2
        pk[64:128, 0:128] = _prep_in_maps._wk2
        pk[0:64, 128:128 + NG * QB] = qTg
        pk[64:128, 128:128 + NT * TK] = kTs
        pk[:, 128 + NG * QB:] = vaug
        in_maps.append({"pk": pk, "cst": cst})
    return in_maps


def kernel(queries, keys, values, valid_lens, W_q, W_k, w_v):
    from concourse.bass_utils import run_bass_kernel_spmd

    _prep_in_maps._wq2 = np.hstack([W_q, W_q]).astype(ml_dtypes.bfloat16)
    _prep_in_maps._wk2 = np.hstack([W_k, W_k]).astype(ml_dtypes.bfloat16)

    plan = _plan(np.asarray(valid_lens))

    if "nc" not in _CACHE:
        _CACHE["nc"] = _build()
    nc = _CACHE["nc"]

    in_maps = _prep_in_maps(queries, keys, values, np.asarray(valid_lens),
                            np.asarray(w_v, dtype=np.float32), plan)
    res = run_bass_kernel_spmd(nc, in_maps, core_ids=list(range(8)))

    num = np.zeros((B, 2, 64, QB), dtype=np.float64)
    den = np.zeros((B, 2, 1, QB), dtype=np.float64)
    for c in range(8):
        po = np.asarray(res.results[c]["po"], dtype=np.float64)  # [72, NG*QB]
        for g, (b, qb, kts) in enumerate(plan[c]):
            if b < 0:
                continue
            sl = po[:, g * QB:(g + 1) * QB]
            num[b, qb] += sl[0:64]
            den[b, qb] += sl[64:65]
    out = num / den  # [B, 2, 64, QB]
    out = out.transpose(0, 1, 3, 2).reshape(B, Q, 64)
    return out.astype(values.dtype)
